# revision 8
# baseline (speedup 1.0000x reference)
"""cosFormer non-causal linear attention on 8 trn2 NeuronCores.

Data-parallel over batch N=8: core b computes batch element b end-to-end.
Per core (L=2048, E=1024, H=16 heads, d=64):
  q = relu(x @ Wq.T), k = relu(x @ Wk.T), v = x @ Wv.T
  q_ = [q*sin, q*cos], k_ = [k*sin, k*cos]    (per-position cos/sin reweight)
  kv_h = k_h^T @ v_h, ksum_h = k_h^T @ 1      (accumulated over L in PSUM)
  attn_h^T = [kv_h | ksum_h]^T @ q_h^T, rows 0:64 scaled by
  z = 1/max(row 64, eps)
  out = attn @ Wo.T

Layouts: activations contract over E_in, so x and all weights enter
transposed (host-side). k_, v live L-major; q_, attn^T live feature-major,
which feeds the output projection without any transposes on device.
All matmuls bf16 with fp32 PSUM accumulation.

PSUM plan (8 banks): A(3 bufs): pk/pq/po, B(2): pv/pa, kvA(1), kvB(1),
ksum(1). kv/ksum accumulate across all of phase 1 with start=False
(banks pre-cleared by a K=1 zero matmul: start=True clears has_written
for the WHOLE bank, so slices sharing a bank must never issue starts).
"""

import numpy as np
import ml_dtypes

import concourse.bass as bass
import concourse.tile as tile
from concourse import bacc, mybir
from concourse.bass_utils import run_bass_kernel_spmd

L, NB, E, H, D = 2048, 8, 1024, 16, 64
KT = E // 128          # 8 contraction tiles
LC = 4                 # L chunks of 512
LCW = L // LC          # 512
LT = L // 128          # 16 l-tiles
EPS = 1e-6

f32 = mybir.dt.float32
bf16 = mybir.dt.bfloat16
AL = mybir.AluOpType
AF = mybir.ActivationFunctionType
BF = np.dtype(ml_dtypes.bfloat16)

_CACHE = {}


def _build():
    if "nc" in _CACHE:
        return _CACHE["nc"]
    nc = bacc.Bacc()

    xt_d = nc.declare_dram_parameter("xt", [E, L], bf16, isOutput=False)
    wq_d = nc.declare_dram_parameter("wq", [E, E], bf16, isOutput=False)
    wk_d = nc.declare_dram_parameter("wk", [E, E], bf16, isOutput=False)
    wv_d = nc.declare_dram_parameter("wv", [E, E], bf16, isOutput=False)
    wo_d = nc.declare_dram_parameter("wo", [E, E], bf16, isOutput=False)
    scol_d = nc.declare_dram_parameter("scol", [128, LT * 2], f32, isOutput=False)
    srow_d = nc.declare_dram_parameter("srow", [128, L], f32, isOutput=False)
    srow2_d = nc.declare_dram_parameter("srow2", [128, L], f32, isOutput=False)
    out_d = nc.declare_dram_parameter("out", [L, E], f32, isOutput=True)
    DBG = _CACHE.get("debug", False)
    if DBG:
        dkv_d = nc.declare_dram_parameter("dkv", [128, 16 * 65], f32, isOutput=True)

    with tile.TileContext(nc) as tc:
        with (
            tc.tile_pool(name="const", bufs=1) as cp,
            tc.tile_pool(name="work", bufs=1) as wp,
            tc.tile_pool(name="ps", bufs=1, space="PSUM") as pp,
        ):
            # ---- resident constants -------------------------------------
            def load_w(dram, name):
                ts = []
                re = dram.rearrange("(t p) e -> t p e", p=128)
                for k in range(KT):
                    t = cp.tile([128, E], bf16, tag=f"{name}{k}", name=f"{name}{k}")
                    nc.sync.dma_start(t[:], re[k])
                    ts.append(t)
                return ts

            xt = []
            xt_re = xt_d.rearrange("(t p) l -> t p l", p=128)
            for k in range(KT):
                t = cp.tile([128, L], bf16, tag=f"xt{k}", name=f"xt{k}")
                nc.sync.dma_start(t[:], xt_re[k])
                xt.append(t)
            wk = load_w(wk_d, "wk")
            wv = load_w(wv_d, "wv")
            wq = load_w(wq_d, "wq")
            wo = load_w(wo_d, "wo")
            scol = cp.tile([128, LT, 2], f32, tag="scol")
            nc.sync.dma_start(scol[:], scol_d.rearrange("p (t c) -> p t c", c=2)[:])
            srow = cp.tile([128, L], f32, tag="srow")
            srow2 = cp.tile([128, L], f32, tag="srow2")
            nc.sync.dma_start(srow[:], srow_d[:])
            nc.sync.dma_start(srow2[:], srow2_d[:])
            ones = cp.tile([128, 1], bf16, tag="ones")
            nc.vector.memset(ones[:], 1.0)

            # kv lhsT tiles in SBUF: 2 groups of 8 heads, (128=2d, 8, 65)
            kv_sb = [wp.tile([128, 8, 65], bf16, tag=f"kv{g}", name=f"kv{g}")
                     for g in range(2)]

            # ---- psum accumulators for phase 1 --------------------------
            kv_ps = [pp.tile([128, 8, 64], f32, tag=f"kvp{g}", name=f"kvp{g}")
                     for g in range(2)]
            ks_ps = pp.tile([128, 16], f32, tag="ksp")
            zl = wp.tile([1, 128], bf16, tag="zl")
            zr5 = wp.tile([1, 512], bf16, tag="zr5")
            nc.vector.memset(zl[:], 0.0)
            nc.vector.memset(zr5[:], 0.0)
            for g in range(2):
                nc.tensor.matmul(kv_ps[g][:].rearrange("p a c -> p (a c)"),
                                 zl[:], zr5[:], start=True, stop=True)
            nc.tensor.matmul(ks_ps[:], zl[:], zr5[:, 0:16], start=True, stop=True)

            # ---- phase 1: kv/ksum accumulation --------------------------
            for lt in range(LT):
                lsl = slice(lt * 128, (lt + 1) * 128)
                for eo in range(2):
                    esl = slice(eo * 512, (eo + 1) * 512)
                    pk = pp.tile([128, 512], f32, tag="A", bufs=3, name=f"pk{lt}_{eo}")
                    pv = pp.tile([128, 512], f32, tag="B", bufs=2, name=f"pv{lt}_{eo}")
                    for k in range(KT):
                        nc.tensor.matmul(pk[:], xt[k][:, lsl], wk[k][:, esl],
                                         start=(k == 0), stop=(k == KT - 1))
                    for k in range(KT):
                        nc.tensor.matmul(pv[:], xt[k][:, lsl], wv[k][:, esl],
                                         start=(k == 0), stop=(k == KT - 1))
                    # k_ build on ACT: per head [0:64]=relu(k)*sin, [64:128]=relu(k)*cos
                    kb = wp.tile([128, 8, 128], bf16, tag="kb", bufs=3,
                                 name=f"kb{lt}_{eo}")
                    pk3 = pk[:].rearrange("p (h e) -> p h e", h=8)
                    nc.scalar.activation(kb[:, :, 0:64], pk3,
                                         AF.Relu, scale=scol[:, lt, 0:1])
                    nc.scalar.activation(kb[:, :, 64:128], pk3,
                                         AF.Relu, scale=scol[:, lt, 1:2])
                    # v copy on ACT
                    vb = wp.tile([128, 8, 64], bf16, tag="vb", bufs=3,
                                 name=f"vb{lt}_{eo}")
                    nc.scalar.activation(vb[:],
                                         pv[:].rearrange("p (h e) -> p h e", h=8),
                                         AF.Copy)
                    for hh in range(8):
                        h = eo * 8 + hh
                        nc.tensor.matmul(kv_ps[h // 8][:, h % 8, :],
                                         kb[:, hh, :], vb[:, hh, :],
                                         start=False, stop=(lt == LT - 1))
                        nc.tensor.matmul(ks_ps[:, h:h + 1],
                                         kb[:, hh, :], ones[:],
                                         start=False, stop=(lt == LT - 1))
            for g in range(2):
                nc.vector.tensor_copy(kv_sb[g][:, :, 0:64], kv_ps[g][:])
                nc.vector.tensor_copy(kv_sb[g][:, :, 64],
                                      ks_ps[:, g * 8:(g + 1) * 8])
            if DBG:
                for g in range(2):
                    nc.gpsimd.dma_start(
                        dkv_d[:, g * 520:(g + 1) * 520],
                        kv_sb[g][:].rearrange("p a b -> p (a b)"))

            # ---- phase 2: q, attention, output projection ---------------
            for lc in range(LC):
                csl = slice(lc * LCW, (lc + 1) * LCW)
                qt = []
                for m in range(KT):
                    pq = pp.tile([128, LCW], f32, tag="A", bufs=3,
                                 name=f"pq{m}_{lc}")
                    for k in range(KT):
                        nc.tensor.matmul(pq[:], wq[k][:, m * 128:(m + 1) * 128],
                                         xt[k][:, csl],
                                         start=(k == 0), stop=(k == KT - 1))
                    for j in range(2):
                        h = 2 * m + j
                        q_h = wp.tile([128, LCW], bf16, tag=f"qt{h}",
                                      name=f"qt{h}_{lc}")
                        rows = slice(j * 64, j * 64 + 64)
                        sin_src = (srow if j == 0 else srow2)[rows, csl]
                        cos_src = (srow2 if j == 0 else srow)[rows, csl]
                        nc.vector.scalar_tensor_tensor(
                            q_h[0:64, :], pq[rows, :], 0.0, sin_src, AL.max, AL.mult)
                        nc.vector.scalar_tensor_tensor(
                            q_h[64:128, :], pq[rows, :], 0.0, cos_src, AL.max, AL.mult)
                        qt.append(q_h)
                at = [wp.tile([128, LCW], bf16, tag=f"at{m}", name=f"at{m}_{lc}",
                              bufs=2) for m in range(KT)]
                for h in range(H):
                    pa = pp.tile([65, LCW], f32, tag="B", bufs=2,
                                 name=f"pa{h}_{lc}")
                    nc.tensor.matmul(pa[:], kv_sb[h // 8][:, h % 8, :], qt[h][:],
                                     start=True, stop=True)
                    zr = wp.tile([1, LCW], f32, tag="zr", bufs=4,
                                 name=f"zr{h}_{lc}")
                    nc.vector.tensor_scalar(zr[:], pa[64:65, :], EPS, None, AL.max)
                    nc.vector.reciprocal_approx_fast(zr[:], zr[:])
                    zb = wp.tile([64, LCW], f32, tag="zb", bufs=4,
                                 name=f"zb{h}_{lc}")
                    nc.gpsimd.partition_broadcast(zb[:], zr[:])
                    rows = slice((h % 2) * 64, (h % 2) * 64 + 64)
                    nc.vector.tensor_tensor(at[h // 2][rows, :], pa[0:64, :],
                                            zb[:], AL.mult)
                for ltl in range(4):
                    lt = lc * 4 + ltl
                    tsl = slice(ltl * 128, (ltl + 1) * 128)
                    for eo in range(2):
                        esl = slice(eo * 512, (eo + 1) * 512)
                        po = pp.tile([128, 512], f32, tag="A", bufs=3,
                                     name=f"po{lt}_{eo}")
                        for m in range(KT):
                            nc.tensor.matmul(po[:], at[m][:, tsl], wo[m][:, esl],
                                             start=(m == 0), stop=(m == KT - 1))
                        ob = wp.tile([128, 512], f32, tag="ob", bufs=3,
                                     name=f"ob{lt}_{eo}")
                        nc.scalar.activation(ob[:], po[:], AF.Copy)
                        nc.gpsimd.dma_start(
                            out_d[lt * 128:(lt + 1) * 128, esl], ob[:])

    nc.compile()
    _CACHE["nc"] = nc
    return nc


def _prep_inputs(query, Wq, Wk, Wv, Wo):
    idx = (np.pi / 2) * np.arange(1, L + 1, dtype=np.float64) / L
    sin = np.sin(idx).astype(np.float32)
    cos = np.cos(idx).astype(np.float32)
    # scol[p, t, c]: c=0 sin, c=1 cos at l = t*128+p
    scol = np.stack([sin.reshape(LT, 128).T, cos.reshape(LT, 128).T],
                    axis=2).reshape(128, LT * 2).copy()
    srow = np.concatenate([np.tile(sin[None, :], (64, 1)),
                           np.tile(cos[None, :], (64, 1))], axis=0).copy()
    srow2 = np.concatenate([srow[64:128], srow[0:64]], axis=0).copy()

    ws = {n: np.ascontiguousarray(w.T).astype(BF)
          for n, w in (("wq", Wq), ("wk", Wk), ("wv", Wv), ("wo", Wo))}
    in_maps = []
    for b in range(NB):
        m = dict(ws)
        m["xt"] = np.ascontiguousarray(query[:, b, :].T).astype(BF)
        m["scol"] = scol
        m["srow"] = srow
        m["srow2"] = srow2
        in_maps.append(m)
    return in_maps


def kernel(query, Wq, Wk, Wv, Wo, _trace=False, _trace_kwargs=None):
    nc = _build()
    in_maps = _prep_inputs(np.asarray(query, np.float32), Wq, Wk, Wv, Wo)
    res = run_bass_kernel_spmd(nc, in_maps, list(range(NB)), trace=_trace,
                               **(_trace_kwargs or {}))
    out = np.stack([res.results[b]["out"] for b in range(NB)], axis=1)
    if _trace:
        kernel.last_result = res
    return np.ascontiguousarray(out, dtype=np.float32)


# revision 9
# speedup vs baseline: 1.0622x; 1.0622x over previous
"""cosFormer non-causal linear attention on 8 trn2 NeuronCores.

Data-parallel over batch N=8: core b computes batch element b end-to-end.
Per core (L=2048, E=1024, H=16 heads, d=64):
  q = relu(x @ Wq.T), k = relu(x @ Wk.T), v = x @ Wv.T
  q_ = [q*sin, q*cos], k_ = [k*sin, k*cos]    (per-position cos/sin reweight)
  kv_h = k_h^T @ v_h, ksum_h = k_h^T @ 1      (accumulated over L in PSUM)
  attn_h^T = [kv_h | ksum_h]^T @ q_h^T, rows 0:64 scaled by
  z = 1/max(row 64, eps)
  out = attn @ Wo.T

Layouts: activations contract over E_in, so x and all weights enter
transposed (host-side). k_, v live L-major; q_, attn^T live feature-major,
which feeds the output projection without any transposes on device.
All matmuls bf16 with fp32 PSUM accumulation.

PSUM plan (8 banks): A(3 bufs): pk/pq/po, B(2): pv/pa, kvA(1), kvB(1),
ksum(1). kv/ksum accumulate across all of phase 1 with start=False
(banks pre-cleared by a K=1 zero matmul: start=True clears has_written
for the WHOLE bank, so slices sharing a bank must never issue starts).
"""

import numpy as np
import ml_dtypes

import concourse.bass as bass
import concourse.tile as tile
from concourse import bacc, mybir
from concourse.bass_utils import run_bass_kernel_spmd

L, NB, E, H, D = 2048, 8, 1024, 16, 64
KT = E // 128          # 8 contraction tiles
LC = 4                 # L chunks of 512
LCW = L // LC          # 512
LT = L // 128          # 16 l-tiles
EPS = 1e-6

f32 = mybir.dt.float32
bf16 = mybir.dt.bfloat16
AL = mybir.AluOpType
AF = mybir.ActivationFunctionType
BF = np.dtype(ml_dtypes.bfloat16)

_CACHE = {}


def _build():
    if "nc" in _CACHE:
        return _CACHE["nc"]
    nc = bacc.Bacc()

    xt_d = nc.declare_dram_parameter("xt", [E, L], bf16, isOutput=False)
    wq_d = nc.declare_dram_parameter("wq", [E, E], bf16, isOutput=False)
    wk_d = nc.declare_dram_parameter("wk", [E, E], bf16, isOutput=False)
    wv_d = nc.declare_dram_parameter("wv", [E, E], bf16, isOutput=False)
    wo_d = nc.declare_dram_parameter("wo", [E, E], bf16, isOutput=False)
    scol_d = nc.declare_dram_parameter("scol", [128, LT * 2], f32, isOutput=False)
    srow_d = nc.declare_dram_parameter("srow", [128, L], f32, isOutput=False)
    srow2_d = nc.declare_dram_parameter("srow2", [128, L], f32, isOutput=False)
    out_d = nc.declare_dram_parameter("out", [L, E], f32, isOutput=True)
    DBG = _CACHE.get("debug", False)
    if DBG:
        dkv_d = nc.declare_dram_parameter("dkv", [128, 16 * 65], f32, isOutput=True)

    with tile.TileContext(nc) as tc:
        with (
            tc.tile_pool(name="const", bufs=1) as cp,
            tc.tile_pool(name="work", bufs=1) as wp,
            tc.tile_pool(name="ps", bufs=1, space="PSUM") as pp,
        ):
            # ---- resident constants -------------------------------------
            def load_w(dram, name):
                ts = []
                re = dram.rearrange("(t p) e -> t p e", p=128)
                for k in range(KT):
                    t = cp.tile([128, E], bf16, tag=f"{name}{k}", name=f"{name}{k}")
                    nc.sync.dma_start(t[:], re[k])
                    ts.append(t)
                return ts

            xt = []
            xt_re = xt_d.rearrange("(t p) l -> t p l", p=128)
            for k in range(KT):
                t = cp.tile([128, L], bf16, tag=f"xt{k}", name=f"xt{k}")
                nc.sync.dma_start(t[:], xt_re[k])
                xt.append(t)
            wk = load_w(wk_d, "wk")
            wv = load_w(wv_d, "wv")
            wq = load_w(wq_d, "wq")
            wo = load_w(wo_d, "wo")
            scol = cp.tile([128, LT, 2], f32, tag="scol")
            nc.sync.dma_start(scol[:], scol_d.rearrange("p (t c) -> p t c", c=2)[:])
            srow = cp.tile([128, L], f32, tag="srow")
            srow2 = cp.tile([128, L], f32, tag="srow2")
            nc.sync.dma_start(srow[:], srow_d[:])
            nc.sync.dma_start(srow2[:], srow2_d[:])
            ones = cp.tile([128, 1], bf16, tag="ones")
            nc.vector.memset(ones[:], 1.0)

            # kv lhsT tiles in SBUF: 4 groups of 4 heads, (128=2d, 4, 65)
            kv_sb = [wp.tile([128, 4, 65], bf16, tag=f"kv{g}", name=f"kv{g}")
                     for g in range(4)]

            # ---- psum accumulators for phase 1 --------------------------
            kv_ps = [pp.tile([128, 4, 65], f32, tag=f"kvp{g}", name=f"kvp{g}")
                     for g in range(4)]
            zl = wp.tile([1, 128], bf16, tag="zl")
            zr5 = wp.tile([1, 260], bf16, tag="zr5")
            nc.vector.memset(zl[:], 0.0)
            nc.vector.memset(zr5[:], 0.0)
            for g in range(4):
                nc.tensor.matmul(kv_ps[g][:].rearrange("p a c -> p (a c)"),
                                 zl[:], zr5[:], start=True, stop=True)

            # ---- phase 1: kv/ksum accumulation --------------------------
            for lt in range(LT):
                lsl = slice(lt * 128, (lt + 1) * 128)
                for eo in range(2):
                    esl = slice(eo * 512, (eo + 1) * 512)
                    pk = pp.tile([128, 512], f32, tag="A", bufs=2, name=f"pk{lt}_{eo}")
                    pv = pp.tile([128, 512], f32, tag="B", bufs=2, name=f"pv{lt}_{eo}")
                    for k in range(KT):
                        nc.tensor.matmul(pk[:], xt[k][:, lsl], wk[k][:, esl],
                                         start=(k == 0), stop=(k == KT - 1))
                    for k in range(KT):
                        nc.tensor.matmul(pv[:], xt[k][:, lsl], wv[k][:, esl],
                                         start=(k == 0), stop=(k == KT - 1))
                    # k_ build on ACT: per head [0:64]=relu(k)*sin, [64:128]=relu(k)*cos
                    kb = wp.tile([128, 8, 128], bf16, tag="kb", bufs=3,
                                 name=f"kb{lt}_{eo}")
                    pk3 = pk[:].rearrange("p (h e) -> p h e", h=8)
                    nc.scalar.activation(kb[:, :, 0:64], pk3,
                                         AF.Relu, scale=scol[:, lt, 0:1])
                    nc.scalar.activation(kb[:, :, 64:128], pk3,
                                         AF.Relu, scale=scol[:, lt, 1:2])
                    # v copy on ACT into 65-wide layout; ones col on DVE
                    vb = wp.tile([128, 8, 65], bf16, tag="vb", bufs=3,
                                 name=f"vb{lt}_{eo}")
                    nc.scalar.activation(vb[:, :, 0:64],
                                         pv[:].rearrange("p (h e) -> p h e", h=8),
                                         AF.Copy)
                    nc.vector.memset(vb[:, :, 64:65], 1.0)
                    for hh in range(8):
                        h = eo * 8 + hh
                        nc.tensor.matmul(kv_ps[h // 4][:, h % 4, :],
                                         kb[:, hh, :], vb[:, hh, :],
                                         start=False, stop=(lt == LT - 1))
            for g in range(4):
                nc.vector.tensor_copy(kv_sb[g][:], kv_ps[g][:])
            if DBG:
                for g in range(4):
                    nc.gpsimd.dma_start(
                        dkv_d[:, g * 260:(g + 1) * 260],
                        kv_sb[g][:].rearrange("p a b -> p (a b)"))

            # ---- phase 2: q, attention, output projection ---------------
            # Emission order pipelines chunks: attn(lc) -> q-proj(lc+1) ->
            # out-proj(lc), so PE has q-projection matmuls to run while the
            # DVE/GpSimd z-chain of chunk lc drains.
            def build_q(lc):
                csl = slice(lc * LCW, (lc + 1) * LCW)
                qts = []
                for m in range(KT):
                    pq = pp.tile([128, LCW], f32, tag="A", bufs=2,
                                 name=f"pq{m}_{lc}")
                    for k in range(KT):
                        nc.tensor.matmul(pq[:], wq[k][:, m * 128:(m + 1) * 128],
                                         xt[k][:, csl],
                                         start=(k == 0), stop=(k == KT - 1))
                    for j in range(2):
                        h = 2 * m + j
                        q_h = wp.tile([128, LCW], bf16, tag=f"qt{h}",
                                      name=f"qt{h}_{lc}")
                        rows = slice(j * 64, j * 64 + 64)
                        sin_src = (srow if j == 0 else srow2)[rows, csl]
                        cos_src = (srow2 if j == 0 else srow)[rows, csl]
                        nc.vector.scalar_tensor_tensor(
                            q_h[0:64, :], pq[rows, :], 0.0, sin_src, AL.max, AL.mult)
                        nc.vector.scalar_tensor_tensor(
                            q_h[64:128, :], pq[rows, :], 0.0, cos_src, AL.max, AL.mult)
                        qts.append(q_h)
                return qts

            qt = build_q(0)
            for lc in range(LC):
                at = [wp.tile([128, LCW], bf16, tag=f"at{m}", name=f"at{m}_{lc}",
                              bufs=2) for m in range(KT)]
                for h in range(H):
                    pa = pp.tile([65, LCW], f32, tag="B", bufs=2,
                                 name=f"pa{h}_{lc}")
                    nc.tensor.matmul(pa[:], kv_sb[h // 4][:, h % 4, :], qt[h][:],
                                     start=True, stop=True)
                    zr = wp.tile([1, LCW], f32, tag="zr", bufs=4,
                                 name=f"zr{h}_{lc}")
                    nc.vector.tensor_scalar(zr[:], pa[64:65, :], EPS, None, AL.max)
                    nc.vector.reciprocal_approx_fast(zr[:], zr[:])
                    zb = wp.tile([64, LCW], f32, tag="zb", bufs=4,
                                 name=f"zb{h}_{lc}")
                    nc.gpsimd.partition_broadcast(zb[:], zr[:])
                    rows = slice((h % 2) * 64, (h % 2) * 64 + 64)
                    nc.vector.tensor_tensor(at[h // 2][rows, :], pa[0:64, :],
                                            zb[:], AL.mult)
                if lc + 1 < LC:
                    qt = build_q(lc + 1)
                for ltl in range(4):
                    lt = lc * 4 + ltl
                    tsl = slice(ltl * 128, (ltl + 1) * 128)
                    for eo in range(2):
                        esl = slice(eo * 512, (eo + 1) * 512)
                        po = pp.tile([128, 512], f32, tag="A", bufs=2,
                                     name=f"po{lt}_{eo}")
                        for m in range(KT):
                            nc.tensor.matmul(po[:], at[m][:, tsl], wo[m][:, esl],
                                             start=(m == 0), stop=(m == KT - 1))
                        ob = wp.tile([128, 512], f32, tag="ob", bufs=3,
                                     name=f"ob{lt}_{eo}")
                        nc.scalar.activation(ob[:], po[:], AF.Copy)
                        nc.gpsimd.dma_start(
                            out_d[lt * 128:(lt + 1) * 128, esl], ob[:])

    nc.compile()
    _CACHE["nc"] = nc
    return nc


def _prep_inputs(query, Wq, Wk, Wv, Wo):
    idx = (np.pi / 2) * np.arange(1, L + 1, dtype=np.float64) / L
    sin = np.sin(idx).astype(np.float32)
    cos = np.cos(idx).astype(np.float32)
    # scol[p, t, c]: c=0 sin, c=1 cos at l = t*128+p
    scol = np.stack([sin.reshape(LT, 128).T, cos.reshape(LT, 128).T],
                    axis=2).reshape(128, LT * 2).copy()
    srow = np.concatenate([np.tile(sin[None, :], (64, 1)),
                           np.tile(cos[None, :], (64, 1))], axis=0).copy()
    srow2 = np.concatenate([srow[64:128], srow[0:64]], axis=0).copy()

    ws = {n: np.ascontiguousarray(w.T).astype(BF)
          for n, w in (("wq", Wq), ("wk", Wk), ("wv", Wv), ("wo", Wo))}
    in_maps = []
    for b in range(NB):
        m = dict(ws)
        m["xt"] = np.ascontiguousarray(query[:, b, :].T).astype(BF)
        m["scol"] = scol
        m["srow"] = srow
        m["srow2"] = srow2
        in_maps.append(m)
    return in_maps


def kernel(query, Wq, Wk, Wv, Wo, _trace=False, _trace_kwargs=None):
    nc = _build()
    in_maps = _prep_inputs(np.asarray(query, np.float32), Wq, Wk, Wv, Wo)
    res = run_bass_kernel_spmd(nc, in_maps, list(range(NB)), trace=_trace,
                               **(_trace_kwargs or {}))
    out = np.stack([res.results[b]["out"] for b in range(NB)], axis=1)
    if _trace:
        kernel.last_result = res
    return np.ascontiguousarray(out, dtype=np.float32)


# revision 11
# speedup vs baseline: 1.4039x; 1.3217x over previous
"""cosFormer non-causal linear attention on 8 trn2 NeuronCores.

Data-parallel over batch N=8: core b computes batch element b end-to-end.
Per core (L=2048, E=1024, H=16 heads, d=64):
  q = relu(x @ Wq.T), k = relu(x @ Wk.T), v = x @ Wv.T
  q_ = [q*sin, q*cos], k_ = [k*sin, k*cos]    (per-position cos/sin reweight)
  kv_h = k_h^T @ v_h, ksum_h = k_h^T @ 1      (accumulated over L in PSUM)
  attn_h^T = [kv_h | ksum_h]^T @ q_h^T, rows 0:64 scaled by
  z = 1/max(row 64, eps)
  out = attn @ Wo.T

Layouts: activations contract over E_in, so x and all weights enter
transposed (host-side). k_, v live L-major; q_, attn^T live feature-major,
which feeds the output projection without any transposes on device.
All matmuls bf16 with fp32 PSUM accumulation.

PSUM plan (8 banks): A(3 bufs): pk/pq/po, B(2): pv/pa, kvA(1), kvB(1),
ksum(1). kv/ksum accumulate across all of phase 1 with start=False
(banks pre-cleared by a K=1 zero matmul: start=True clears has_written
for the WHOLE bank, so slices sharing a bank must never issue starts).
"""

import numpy as np
import ml_dtypes

import concourse.bass as bass
import concourse.tile as tile
from concourse import bacc, mybir
from concourse.bass_utils import run_bass_kernel_spmd

L, NB, E, H, D = 2048, 8, 1024, 16, 64
KT = E // 128          # 8 contraction tiles
LC = 4                 # L chunks of 512
LCW = L // LC          # 512
LT = L // 128          # 16 l-tiles
EPS = 1e-6

f32 = mybir.dt.float32
bf16 = mybir.dt.bfloat16
AL = mybir.AluOpType
AF = mybir.ActivationFunctionType
BF = np.dtype(ml_dtypes.bfloat16)

_CACHE = {}


def _build():
    if "nc" in _CACHE:
        return _CACHE["nc"]
    nc = bacc.Bacc()

    xt_d = nc.declare_dram_parameter("xt", [E, L], bf16, isOutput=False)
    wq_d = nc.declare_dram_parameter("wq", [E, E], bf16, isOutput=False)
    wk_d = nc.declare_dram_parameter("wk", [E, E], bf16, isOutput=False)
    wv_d = nc.declare_dram_parameter("wv", [E, E], bf16, isOutput=False)
    wo_d = nc.declare_dram_parameter("wo", [E, E], bf16, isOutput=False)
    scol_d = nc.declare_dram_parameter("scol", [128, LT * 2], f32, isOutput=False)
    srow_d = nc.declare_dram_parameter("srow", [128, L], bf16, isOutput=False)
    srow2_d = nc.declare_dram_parameter("srow2", [128, L], bf16, isOutput=False)
    out_d = nc.declare_dram_parameter("out", [L, E], f32, isOutput=True)
    DBG = _CACHE.get("debug", False)
    if DBG:
        dkv_d = nc.declare_dram_parameter("dkv", [128, 16 * 65], f32, isOutput=True)

    with tile.TileContext(nc) as tc:
        with (
            tc.tile_pool(name="const", bufs=1) as cp,
            tc.tile_pool(name="work", bufs=1) as wp,
            tc.tile_pool(name="ps", bufs=1, space="PSUM") as pp,
        ):
            # ---- resident constants -------------------------------------
            def load_w(dram, name):
                ts = []
                re = dram.rearrange("(t p) e -> t p e", p=128)
                for k in range(KT):
                    t = cp.tile([128, E], bf16, tag=f"{name}{k}", name=f"{name}{k}")
                    nc.sync.dma_start(t[:], re[k])
                    ts.append(t)
                return ts

            xt = []
            xt_re = xt_d.rearrange("(t p) l -> t p l", p=128)
            for k in range(KT):
                t = cp.tile([128, L], bf16, tag=f"xt{k}", name=f"xt{k}")
                nc.sync.dma_start(t[:], xt_re[k])
                xt.append(t)
            wk = load_w(wk_d, "wk")
            wv = load_w(wv_d, "wv")
            wq = load_w(wq_d, "wq")
            wo = load_w(wo_d, "wo")
            scol = cp.tile([128, LT, 2], f32, tag="scol")
            nc.sync.dma_start(scol[:], scol_d.rearrange("p (t c) -> p t c", c=2)[:])
            srow = cp.tile([128, L], bf16, tag="srow")
            srow2 = cp.tile([128, L], bf16, tag="srow2")
            nc.sync.dma_start(srow[:], srow_d[:])
            nc.sync.dma_start(srow2[:], srow2_d[:])
            ones = cp.tile([128, 1], bf16, tag="ones")
            nc.vector.memset(ones[:], 1.0)
            epsc = cp.tile([128, 1], f32, tag="epsc")
            nc.vector.memset(epsc[:], EPS)

            # kv lhsT tiles in SBUF: 4 groups of 4 heads, (128=2d, 4, 65)
            kv_sb = [wp.tile([128, 4, 65], bf16, tag=f"kv{g}", name=f"kv{g}")
                     for g in range(4)]

            # ---- psum accumulators for phase 1 --------------------------
            kv_ps = [pp.tile([128, 4, 65], f32, tag=f"kvp{g}", name=f"kvp{g}")
                     for g in range(4)]
            zl = wp.tile([1, 128], bf16, tag="zl")
            zr5 = wp.tile([1, 260], bf16, tag="zr5")
            nc.vector.memset(zl[:], 0.0)
            nc.vector.memset(zr5[:], 0.0)
            for g in range(4):
                nc.tensor.matmul(kv_ps[g][:].rearrange("p a c -> p (a c)"),
                                 zl[:], zr5[:], start=True, stop=True)

            # ---- phase 1: kv/ksum accumulation --------------------------
            for lt in range(LT):
                lsl = slice(lt * 128, (lt + 1) * 128)
                for eo in range(2):
                    esl = slice(eo * 512, (eo + 1) * 512)
                    pk = pp.tile([128, 512], f32, tag="A", bufs=2, name=f"pk{lt}_{eo}")
                    pv = pp.tile([128, 512], f32, tag="B", bufs=2, name=f"pv{lt}_{eo}")
                    for k in range(KT):
                        nc.tensor.matmul(pk[:], xt[k][:, lsl], wk[k][:, esl],
                                         start=(k == 0), stop=(k == KT - 1))
                    for k in range(KT):
                        nc.tensor.matmul(pv[:], xt[k][:, lsl], wv[k][:, esl],
                                         start=(k == 0), stop=(k == KT - 1))
                    # k_ build on ACT: per head [0:64]=relu(k)*sin, [64:128]=relu(k)*cos
                    kb = wp.tile([128, 8, 128], bf16, tag="kb", bufs=3,
                                 name=f"kb{lt}_{eo}")
                    pk3 = pk[:].rearrange("p (h e) -> p h e", h=8)
                    nc.scalar.activation(kb[:, :, 0:64], pk3,
                                         AF.Relu, scale=scol[:, lt, 0:1])
                    nc.scalar.activation(kb[:, :, 64:128], pk3,
                                         AF.Relu, scale=scol[:, lt, 1:2])
                    # v copy on ACT into 65-wide layout; ones col on DVE
                    vb = wp.tile([128, 8, 65], bf16, tag="vb", bufs=3,
                                 name=f"vb{lt}_{eo}")
                    nc.scalar.activation(vb[:, :, 0:64],
                                         pv[:].rearrange("p (h e) -> p h e", h=8),
                                         AF.Copy)
                    nc.vector.memset(vb[:, :, 64:65], 1.0)
                    for hh in range(8):
                        h = eo * 8 + hh
                        nc.tensor.matmul(kv_ps[h // 4][:, h % 4, :],
                                         kb[:, hh, :], vb[:, hh, :],
                                         start=False, stop=(lt == LT - 1))
            for g in range(4):
                nc.vector.tensor_copy(kv_sb[g][:], kv_ps[g][:])
            if DBG:
                for g in range(4):
                    nc.gpsimd.dma_start(
                        dkv_d[:, g * 260:(g + 1) * 260],
                        kv_sb[g][:].rearrange("p a b -> p (a b)"))

            # ---- phase 2: q, attention, output projection ---------------
            # Emission order pipelines chunks: attn(lc) -> q-proj(lc+1) ->
            # out-proj(lc), so PE has q-projection matmuls to run while the
            # DVE/GpSimd z-chain of chunk lc drains.
            def build_q(lc):
                csl = slice(lc * LCW, (lc + 1) * LCW)
                qts = []
                for m in range(KT):
                    pq = pp.tile([128, LCW], f32, tag="A", bufs=2,
                                 name=f"pq{m}_{lc}")
                    for k in range(KT):
                        nc.tensor.matmul(pq[:], wq[k][:, m * 128:(m + 1) * 128],
                                         xt[k][:, csl],
                                         start=(k == 0), stop=(k == KT - 1))
                    qr = wp.tile([128, LCW], bf16, tag="qr", bufs=3,
                                 name=f"qr{m}_{lc}")
                    nc.scalar.activation(qr[:], pq[:], AF.Relu)
                    for j in range(2):
                        h = 2 * m + j
                        q_h = wp.tile([128, LCW], bf16, tag=f"qt{h}",
                                      name=f"qt{h}_{lc}")
                        rows = slice(j * 64, j * 64 + 64)
                        sin_src = (srow if j == 0 else srow2)[rows, csl]
                        cos_src = (srow2 if j == 0 else srow)[rows, csl]
                        nc.vector.tensor_tensor(q_h[0:64, :], qr[rows, :],
                                                sin_src, AL.mult)
                        nc.vector.tensor_tensor(q_h[64:128, :], qr[rows, :],
                                                cos_src, AL.mult)
                        qts.append(q_h)
                return qts

            qt = build_q(0)
            for lc in range(LC):
                at = [wp.tile([128, LCW], bf16, tag=f"at{m}", name=f"at{m}_{lc}",
                              bufs=2) for m in range(KT)]
                for h in range(H):
                    pa = pp.tile([65, LCW], f32, tag=f"kvp{h % 4}",
                                 name=f"pa{h}_{lc}")
                    nc.tensor.matmul(pa[:], kv_sb[h // 4][:, h % 4, :], qt[h][:],
                                     start=True, stop=True)
                    zr = wp.tile([1, LCW], f32, tag="zr", bufs=8,
                                 name=f"zr{h}_{lc}")
                    # z = 1/(x + eps) instead of 1/max(x, eps): x >= 0, and
                    # where they differ (x ~ eps) the numerator is ~0 anyway.
                    nc.scalar.activation(zr[:], pa[64:65, :], AF.Identity,
                                         bias=epsc[64:65, :])
                    nc.vector.reciprocal_approx_fast(zr[:], zr[:])
                    zb = wp.tile([64, LCW], f32, tag="zb", bufs=8,
                                 name=f"zb{h}_{lc}")
                    nc.gpsimd.partition_broadcast(zb[:], zr[:])
                    rows = slice((h % 2) * 64, (h % 2) * 64 + 64)
                    nc.vector.tensor_tensor(at[h // 2][rows, :], pa[0:64, :],
                                            zb[:], AL.mult)
                if lc + 1 < LC:
                    qt = build_q(lc + 1)
                for ltl in range(4):
                    lt = lc * 4 + ltl
                    tsl = slice(ltl * 128, (ltl + 1) * 128)
                    for eo in range(2):
                        esl = slice(eo * 512, (eo + 1) * 512)
                        po = pp.tile([128, 512], f32, tag="B", bufs=2,
                                     name=f"po{lt}_{eo}")
                        for m in range(KT):
                            nc.tensor.matmul(po[:], at[m][:, tsl], wo[m][:, esl],
                                             start=(m == 0), stop=(m == KT - 1))
                        ob = wp.tile([128, 512], f32, tag="ob", bufs=3,
                                     name=f"ob{lt}_{eo}")
                        nc.scalar.activation(ob[:], po[:], AF.Copy)
                        nc.gpsimd.dma_start(
                            out_d[lt * 128:(lt + 1) * 128, esl], ob[:])

    nc.compile()
    _CACHE["nc"] = nc
    return nc


def _prep_inputs(query, Wq, Wk, Wv, Wo):
    idx = (np.pi / 2) * np.arange(1, L + 1, dtype=np.float64) / L
    sin = np.sin(idx).astype(np.float32)
    cos = np.cos(idx).astype(np.float32)
    # scol[p, t, c]: c=0 sin, c=1 cos at l = t*128+p
    scol = np.stack([sin.reshape(LT, 128).T, cos.reshape(LT, 128).T],
                    axis=2).reshape(128, LT * 2).copy()
    srow = np.concatenate([np.tile(sin[None, :], (64, 1)),
                           np.tile(cos[None, :], (64, 1))], axis=0).astype(BF)
    srow2 = np.concatenate([srow[64:128], srow[0:64]]).copy()

    ws = {n: np.ascontiguousarray(w.T).astype(BF)
          for n, w in (("wq", Wq), ("wk", Wk), ("wv", Wv), ("wo", Wo))}
    in_maps = []
    for b in range(NB):
        m = dict(ws)
        m["xt"] = np.ascontiguousarray(query[:, b, :].T).astype(BF)
        m["scol"] = scol
        m["srow"] = srow
        m["srow2"] = srow2
        in_maps.append(m)
    return in_maps


def kernel(query, Wq, Wk, Wv, Wo, _trace=False, _trace_kwargs=None):
    nc = _build()
    in_maps = _prep_inputs(np.asarray(query, np.float32), Wq, Wk, Wv, Wo)
    res = run_bass_kernel_spmd(nc, in_maps, list(range(NB)), trace=_trace,
                               **(_trace_kwargs or {}))
    out = np.stack([res.results[b]["out"] for b in range(NB)], axis=1)
    if _trace:
        kernel.last_result = res
    return np.ascontiguousarray(out, dtype=np.float32)


# revision 12
# speedup vs baseline: 1.4289x; 1.0178x over previous
"""cosFormer non-causal linear attention on 8 trn2 NeuronCores.

Data-parallel over batch N=8: core b computes batch element b end-to-end.
Per core (L=2048, E=1024, H=16 heads, d=64):
  q = relu(x @ Wq.T), k = relu(x @ Wk.T), v = x @ Wv.T
  q_ = [q*sin, q*cos], k_ = [k*sin, k*cos]    (per-position cos/sin reweight)
  kv_h = k_h^T @ v_h, ksum_h = k_h^T @ 1      (accumulated over L in PSUM)
  attn_h^T = [kv_h | ksum_h]^T @ q_h^T, rows 0:64 scaled by
  z = 1/max(row 64, eps)
  out = attn @ Wo.T

Layouts: activations contract over E_in, so x and all weights enter
transposed (host-side). k_, v live L-major; q_, attn^T live feature-major,
which feeds the output projection without any transposes on device.
All matmuls bf16 with fp32 PSUM accumulation.

PSUM plan (8 banks): A(3 bufs): pk/pq/po, B(2): pv/pa, kvA(1), kvB(1),
ksum(1). kv/ksum accumulate across all of phase 1 with start=False
(banks pre-cleared by a K=1 zero matmul: start=True clears has_written
for the WHOLE bank, so slices sharing a bank must never issue starts).
"""

import numpy as np
import ml_dtypes

import concourse.bass as bass
import concourse.tile as tile
from concourse import bacc, mybir
from concourse.bass_utils import run_bass_kernel_spmd

L, NB, E, H, D = 2048, 8, 1024, 16, 64
KT = E // 128          # 8 contraction tiles
LC = 4                 # L chunks of 512
LCW = L // LC          # 512
LT = L // 128          # 16 l-tiles
EPS = 1e-6

f32 = mybir.dt.float32
bf16 = mybir.dt.bfloat16
AL = mybir.AluOpType
AF = mybir.ActivationFunctionType
BF = np.dtype(ml_dtypes.bfloat16)

_CACHE = {}


def _build():
    if "nc" in _CACHE:
        return _CACHE["nc"]
    nc = bacc.Bacc()

    xt_d = nc.declare_dram_parameter("xt", [E, L], bf16, isOutput=False)
    wq_d = nc.declare_dram_parameter("wq", [E, E], bf16, isOutput=False)
    wk_d = nc.declare_dram_parameter("wk", [E, E], bf16, isOutput=False)
    wv_d = nc.declare_dram_parameter("wv", [E, E], bf16, isOutput=False)
    wo_d = nc.declare_dram_parameter("wo", [E, E], bf16, isOutput=False)
    scol_d = nc.declare_dram_parameter("scol", [128, LT * 2], f32, isOutput=False)
    srow_d = nc.declare_dram_parameter("srow", [128, L], bf16, isOutput=False)
    srow2_d = nc.declare_dram_parameter("srow2", [128, L], bf16, isOutput=False)
    out_d = nc.declare_dram_parameter("out", [L, E], f32, isOutput=True)
    DBG = _CACHE.get("debug", False)
    if DBG:
        dkv_d = nc.declare_dram_parameter("dkv", [128, 16 * 65], f32, isOutput=True)

    with tile.TileContext(nc) as tc:
        with (
            tc.tile_pool(name="const", bufs=1) as cp,
            tc.tile_pool(name="work", bufs=1) as wp,
            tc.tile_pool(name="ps", bufs=1, space="PSUM") as pp,
        ):
            # ---- resident constants -------------------------------------
            # DMA order matches first-use order: scol first (first ACT),
            # then per k: wk_k, xt_k, wv_k so projection matmul k can start
            # as soon as its own operands land.
            scol = cp.tile([128, LT, 2], f32, tag="scol")
            nc.sync.dma_start(scol[:], scol_d.rearrange("p (t c) -> p t c", c=2)[:])
            xt, wk, wv, wq, wo = [], [], [], [], []
            res = {n: d.rearrange("(t p) e -> t p e", p=128)
                   for n, d in (("wk", wk_d), ("wv", wv_d), ("wq", wq_d),
                                ("wo", wo_d))}
            xt_re = xt_d.rearrange("(t p) l -> t p l", p=128)
            for k in range(KT):
                tw = cp.tile([128, E], bf16, tag=f"wk{k}", name=f"wk{k}")
                nc.sync.dma_start(tw[:], res["wk"][k])
                wk.append(tw)
                t = cp.tile([128, L], bf16, tag=f"xt{k}", name=f"xt{k}")
                nc.sync.dma_start(t[:], xt_re[k])
                xt.append(t)
                tv = cp.tile([128, E], bf16, tag=f"wv{k}", name=f"wv{k}")
                nc.sync.dma_start(tv[:], res["wv"][k])
                wv.append(tv)
            for k in range(KT):
                tq = cp.tile([128, E], bf16, tag=f"wq{k}", name=f"wq{k}")
                nc.sync.dma_start(tq[:], res["wq"][k])
                wq.append(tq)
            for k in range(KT):
                to = cp.tile([128, E], bf16, tag=f"wo{k}", name=f"wo{k}")
                nc.sync.dma_start(to[:], res["wo"][k])
                wo.append(to)
            srow = cp.tile([128, L], bf16, tag="srow")
            srow2 = cp.tile([128, L], bf16, tag="srow2")
            nc.sync.dma_start(srow[:], srow_d[:])
            nc.sync.dma_start(srow2[:], srow2_d[:])
            ones = cp.tile([128, 1], bf16, tag="ones")
            nc.vector.memset(ones[:], 1.0)
            epsc = cp.tile([128, 1], f32, tag="epsc")
            nc.vector.memset(epsc[:], EPS)

            # kv lhsT tiles in SBUF: 4 groups of 4 heads, (128=2d, 4, 65)
            kv_sb = [wp.tile([128, 4, 65], bf16, tag=f"kv{g}", name=f"kv{g}")
                     for g in range(4)]

            # ---- psum accumulators for phase 1 --------------------------
            kv_ps = [pp.tile([128, 4, 65], f32, tag=f"kvp{g}", name=f"kvp{g}")
                     for g in range(4)]
            zl = wp.tile([1, 128], bf16, tag="zl")
            zr5 = wp.tile([1, 260], bf16, tag="zr5")
            nc.vector.memset(zl[:], 0.0)
            nc.vector.memset(zr5[:], 0.0)
            for g in range(4):
                nc.tensor.matmul(kv_ps[g][:].rearrange("p a c -> p (a c)"),
                                 zl[:], zr5[:], start=True, stop=True)

            # ---- phase 1: kv/ksum accumulation --------------------------
            for lt in range(LT):
                lsl = slice(lt * 128, (lt + 1) * 128)
                for eo in range(2):
                    esl = slice(eo * 512, (eo + 1) * 512)
                    pk = pp.tile([128, 512], f32, tag="A", bufs=2, name=f"pk{lt}_{eo}")
                    pv = pp.tile([128, 512], f32, tag="B", bufs=2, name=f"pv{lt}_{eo}")
                    for k in range(KT):
                        nc.tensor.matmul(pk[:], xt[k][:, lsl], wk[k][:, esl],
                                         start=(k == 0), stop=(k == KT - 1))
                    for k in range(KT):
                        nc.tensor.matmul(pv[:], xt[k][:, lsl], wv[k][:, esl],
                                         start=(k == 0), stop=(k == KT - 1))
                    # k_ build on ACT: per head [0:64]=relu(k)*sin, [64:128]=relu(k)*cos
                    kb = wp.tile([128, 8, 128], bf16, tag="kb", bufs=3,
                                 name=f"kb{lt}_{eo}")
                    pk3 = pk[:].rearrange("p (h e) -> p h e", h=8)
                    nc.scalar.activation(kb[:, :, 0:64], pk3,
                                         AF.Relu, scale=scol[:, lt, 0:1])
                    nc.scalar.activation(kb[:, :, 64:128], pk3,
                                         AF.Relu, scale=scol[:, lt, 1:2])
                    # v copy on ACT into 65-wide layout; ones col on DVE
                    vb = wp.tile([128, 8, 65], bf16, tag="vb", bufs=3,
                                 name=f"vb{lt}_{eo}")
                    nc.scalar.activation(vb[:, :, 0:64],
                                         pv[:].rearrange("p (h e) -> p h e", h=8),
                                         AF.Copy)
                    nc.vector.memset(vb[:, :, 64:65], 1.0)
                    for hh in range(8):
                        h = eo * 8 + hh
                        nc.tensor.matmul(kv_ps[h // 4][:, h % 4, :],
                                         kb[:, hh, :], vb[:, hh, :],
                                         start=False, stop=(lt == LT - 1))
            for g in range(4):
                nc.vector.tensor_copy(kv_sb[g][:], kv_ps[g][:])
            if DBG:
                for g in range(4):
                    nc.gpsimd.dma_start(
                        dkv_d[:, g * 260:(g + 1) * 260],
                        kv_sb[g][:].rearrange("p a b -> p (a b)"))

            # ---- phase 2: q, attention, output projection ---------------
            # Emission order pipelines chunks: attn(lc) -> q-proj(lc+1) ->
            # out-proj(lc), so PE has q-projection matmuls to run while the
            # DVE/GpSimd z-chain of chunk lc drains.
            def build_q(lc):
                csl = slice(lc * LCW, (lc + 1) * LCW)
                qts = []
                for m in range(KT):
                    pq = pp.tile([128, LCW], f32, tag="A", bufs=2,
                                 name=f"pq{m}_{lc}")
                    for k in range(KT):
                        nc.tensor.matmul(pq[:], wq[k][:, m * 128:(m + 1) * 128],
                                         xt[k][:, csl],
                                         start=(k == 0), stop=(k == KT - 1))
                    qr = wp.tile([128, LCW], bf16, tag="qr", bufs=3,
                                 name=f"qr{m}_{lc}")
                    nc.scalar.activation(qr[:], pq[:], AF.Relu)
                    for j in range(2):
                        h = 2 * m + j
                        q_h = wp.tile([128, LCW], bf16, tag=f"qt{h}",
                                      name=f"qt{h}_{lc}")
                        rows = slice(j * 64, j * 64 + 64)
                        sin_src = (srow if j == 0 else srow2)[rows, csl]
                        cos_src = (srow2 if j == 0 else srow)[rows, csl]
                        nc.vector.tensor_tensor(q_h[0:64, :], qr[rows, :],
                                                sin_src, AL.mult)
                        nc.vector.tensor_tensor(q_h[64:128, :], qr[rows, :],
                                                cos_src, AL.mult)
                        qts.append(q_h)
                return qts

            qt = build_q(0)
            for lc in range(LC):
                at = [wp.tile([128, LCW], bf16, tag=f"at{m}", name=f"at{m}_{lc}",
                              bufs=2) for m in range(KT)]
                for h in range(H):
                    pa = pp.tile([65, LCW], f32, tag=f"kvp{h % 4}",
                                 name=f"pa{h}_{lc}")
                    nc.tensor.matmul(pa[:], kv_sb[h // 4][:, h % 4, :], qt[h][:],
                                     start=True, stop=True)
                    zr = wp.tile([1, LCW], f32, tag="zr", bufs=8,
                                 name=f"zr{h}_{lc}")
                    # z = 1/(x + eps) instead of 1/max(x, eps): x >= 0, and
                    # where they differ (x ~ eps) the numerator is ~0 anyway.
                    nc.scalar.activation(zr[:], pa[64:65, :], AF.Identity,
                                         bias=epsc[64:65, :])
                    nc.vector.reciprocal_approx_fast(zr[:], zr[:])
                    zb = wp.tile([64, LCW], f32, tag="zb", bufs=8,
                                 name=f"zb{h}_{lc}")
                    nc.gpsimd.partition_broadcast(zb[:], zr[:])
                    rows = slice((h % 2) * 64, (h % 2) * 64 + 64)
                    nc.vector.tensor_tensor(at[h // 2][rows, :], pa[0:64, :],
                                            zb[:], AL.mult)
                if lc + 1 < LC:
                    qt = build_q(lc + 1)
                for ltl in range(4):
                    lt = lc * 4 + ltl
                    tsl = slice(ltl * 128, (ltl + 1) * 128)
                    for eo in range(2):
                        esl = slice(eo * 512, (eo + 1) * 512)
                        po = pp.tile([128, 512], f32, tag="B", bufs=2,
                                     name=f"po{lt}_{eo}")
                        for m in range(KT):
                            nc.tensor.matmul(po[:], at[m][:, tsl], wo[m][:, esl],
                                             start=(m == 0), stop=(m == KT - 1))
                        ob = wp.tile([128, 512], f32, tag="ob", bufs=3,
                                     name=f"ob{lt}_{eo}")
                        nc.scalar.activation(ob[:], po[:], AF.Copy)
                        nc.gpsimd.dma_start(
                            out_d[lt * 128:(lt + 1) * 128, esl], ob[:])

    nc.compile()
    _CACHE["nc"] = nc
    return nc


def _prep_inputs(query, Wq, Wk, Wv, Wo):
    idx = (np.pi / 2) * np.arange(1, L + 1, dtype=np.float64) / L
    sin = np.sin(idx).astype(np.float32)
    cos = np.cos(idx).astype(np.float32)
    # scol[p, t, c]: c=0 sin, c=1 cos at l = t*128+p
    scol = np.stack([sin.reshape(LT, 128).T, cos.reshape(LT, 128).T],
                    axis=2).reshape(128, LT * 2).copy()
    srow = np.concatenate([np.tile(sin[None, :], (64, 1)),
                           np.tile(cos[None, :], (64, 1))], axis=0).astype(BF)
    srow2 = np.concatenate([srow[64:128], srow[0:64]]).copy()

    ws = {n: np.ascontiguousarray(w.T).astype(BF)
          for n, w in (("wq", Wq), ("wk", Wk), ("wv", Wv), ("wo", Wo))}
    in_maps = []
    for b in range(NB):
        m = dict(ws)
        m["xt"] = np.ascontiguousarray(query[:, b, :].T).astype(BF)
        m["scol"] = scol
        m["srow"] = srow
        m["srow2"] = srow2
        in_maps.append(m)
    return in_maps


def kernel(query, Wq, Wk, Wv, Wo, _trace=False, _trace_kwargs=None):
    nc = _build()
    in_maps = _prep_inputs(np.asarray(query, np.float32), Wq, Wk, Wv, Wo)
    res = run_bass_kernel_spmd(nc, in_maps, list(range(NB)), trace=_trace,
                               **(_trace_kwargs or {}))
    out = np.stack([res.results[b]["out"] for b in range(NB)], axis=1)
    if _trace:
        kernel.last_result = res
    return np.ascontiguousarray(out, dtype=np.float32)


# revision 15
# speedup vs baseline: 1.4424x; 1.0094x over previous
"""cosFormer non-causal linear attention on 8 trn2 NeuronCores.

Data-parallel over batch N=8: core b computes batch element b end-to-end.
Per core (L=2048, E=1024, H=16 heads, d=64):
  q = relu(x @ Wq.T), k = relu(x @ Wk.T), v = x @ Wv.T
  q_ = [q*sin, q*cos], k_ = [k*sin, k*cos]    (per-position cos/sin reweight)
  kv_h = k_h^T @ v_h, ksum_h = k_h^T @ 1      (accumulated over L in PSUM)
  attn_h^T = [kv_h | ksum_h]^T @ q_h^T, rows 0:64 scaled by
  z = 1/max(row 64, eps)
  out = attn @ Wo.T

Layouts: activations contract over E_in, so x and all weights enter
transposed (host-side). k_, v live L-major; q_, attn^T live feature-major,
which feeds the output projection without any transposes on device.
All matmuls bf16 with fp32 PSUM accumulation.

PSUM plan (8 banks): A(3 bufs): pk/pq/po, B(2): pv/pa, kvA(1), kvB(1),
ksum(1). kv/ksum accumulate across all of phase 1 with start=False
(banks pre-cleared by a K=1 zero matmul: start=True clears has_written
for the WHOLE bank, so slices sharing a bank must never issue starts).
"""

import numpy as np
import ml_dtypes

import concourse.bass as bass
import concourse.tile as tile
from concourse import bacc, mybir
from concourse.bass_utils import run_bass_kernel_spmd

L, NB, E, H, D = 2048, 8, 1024, 16, 64
KT = E // 128          # 8 contraction tiles
LC = 4                 # L chunks of 512
LCW = L // LC          # 512
LT = L // 128          # 16 l-tiles
EPS = 1e-6

f32 = mybir.dt.float32
bf16 = mybir.dt.bfloat16
AL = mybir.AluOpType
AF = mybir.ActivationFunctionType
BF = np.dtype(ml_dtypes.bfloat16)

_CACHE = {}


def _build():
    if "nc" in _CACHE:
        return _CACHE["nc"]
    nc = bacc.Bacc()

    xt_d = nc.declare_dram_parameter("xt", [E, L], bf16, isOutput=False)
    wq_d = nc.declare_dram_parameter("wq", [E, E], bf16, isOutput=False)
    wk_d = nc.declare_dram_parameter("wk", [E, E], bf16, isOutput=False)
    wv_d = nc.declare_dram_parameter("wv", [E, E], bf16, isOutput=False)
    wo_d = nc.declare_dram_parameter("wo", [E, E], bf16, isOutput=False)
    scol_d = nc.declare_dram_parameter("scol", [128, LT * 2], f32, isOutput=False)
    srow_d = nc.declare_dram_parameter("srow", [128, L], bf16, isOutput=False)
    srow2_d = nc.declare_dram_parameter("srow2", [128, L], bf16, isOutput=False)
    out_d = nc.declare_dram_parameter("out", [L, E], f32, isOutput=True)
    DBG = _CACHE.get("debug", False)
    if DBG:
        dkv_d = nc.declare_dram_parameter("dkv", [128, 16 * 65], f32, isOutput=True)

    with tile.TileContext(nc) as tc:
        with (
            tc.tile_pool(name="const", bufs=1) as cp,
            tc.tile_pool(name="work", bufs=1) as wp,
            tc.tile_pool(name="ps", bufs=1, space="PSUM") as pp,
        ):
            # ---- resident constants -------------------------------------
            # DMA order matches first-use order: scol first (first ACT),
            # then per k: wk_k, xt_k, wv_k so projection matmul k can start
            # as soon as its own operands land.
            scol = cp.tile([128, LT, 2], f32, tag="scol")
            nc.sync.dma_start(scol[:], scol_d.rearrange("p (t c) -> p t c", c=2)[:])
            xt, wk, wv, wq, wo = [], [], [], [], []
            res = {n: d.rearrange("(t p) e -> t p e", p=128)
                   for n, d in (("wk", wk_d), ("wv", wv_d), ("wq", wq_d),
                                ("wo", wo_d))}
            xt_re = xt_d.rearrange("(t p) l -> t p l", p=128)
            for k in range(KT):
                tw = cp.tile([128, E], bf16, tag=f"wk{k}", name=f"wk{k}")
                nc.sync.dma_start(tw[:], res["wk"][k])
                wk.append(tw)
                t = cp.tile([128, L], bf16, tag=f"xt{k}", name=f"xt{k}")
                nc.sync.dma_start(t[:], xt_re[k])
                xt.append(t)
                tv = cp.tile([128, E], bf16, tag=f"wv{k}", name=f"wv{k}")
                nc.sync.dma_start(tv[:], res["wv"][k])
                wv.append(tv)
            for k in range(KT):
                tq = cp.tile([128, E], bf16, tag=f"wq{k}", name=f"wq{k}")
                nc.sync.dma_start(tq[:], res["wq"][k])
                wq.append(tq)
            for k in range(KT):
                to = cp.tile([128, E], bf16, tag=f"wo{k}", name=f"wo{k}")
                nc.sync.dma_start(to[:], res["wo"][k])
                wo.append(to)
            srow = cp.tile([128, L], bf16, tag="srow")
            srow2 = cp.tile([128, L], bf16, tag="srow2")
            nc.sync.dma_start(srow[:], srow_d[:])
            nc.sync.dma_start(srow2[:], srow2_d[:])
            ones = cp.tile([128, 1], bf16, tag="ones")
            nc.vector.memset(ones[:], 1.0)
            epsc = cp.tile([128, 1], f32, tag="epsc")
            nc.vector.memset(epsc[:], EPS)

            # kv lhsT tiles in SBUF: 4 groups of 4 heads, (128=2d, 4, 65)
            kv_sb = [wp.tile([128, 4, 65], bf16, tag=f"kv{g}", name=f"kv{g}")
                     for g in range(4)]

            # ---- psum accumulators for phase 1 --------------------------
            kv_ps = [pp.tile([128, 4, 65], f32, tag=f"kvp{g}", name=f"kvp{g}")
                     for g in range(4)]
            zl = wp.tile([1, 128], bf16, tag="zl")
            zr5 = wp.tile([1, 260], bf16, tag="zr5")
            nc.vector.memset(zl[:], 0.0)
            nc.vector.memset(zr5[:], 0.0)
            for g in range(4):
                nc.tensor.matmul(kv_ps[g][:].rearrange("p a c -> p (a c)"),
                                 zl[:], zr5[:], start=True, stop=True)

            # ---- phase 1: kv/ksum accumulation --------------------------
            for lt in range(LT):
                lsl = slice(lt * 128, (lt + 1) * 128)
                for eo in range(2):
                    esl = slice(eo * 512, (eo + 1) * 512)
                    pk = pp.tile([128, 512], f32, tag="A", bufs=2, name=f"pk{lt}_{eo}")
                    pv = pp.tile([128, 512], f32, tag="B", bufs=2, name=f"pv{lt}_{eo}")
                    for k in range(KT):
                        nc.tensor.matmul(pk[:], xt[k][:, lsl], wk[k][:, esl],
                                         start=(k == 0), stop=(k == KT - 1))
                    for k in range(KT):
                        nc.tensor.matmul(pv[:], xt[k][:, lsl], wv[k][:, esl],
                                         start=(k == 0), stop=(k == KT - 1))
                    # k_ build on ACT: per head [0:64]=relu(k)*sin, [64:128]=relu(k)*cos
                    kb = wp.tile([128, 8, 128], bf16, tag="kb", bufs=3,
                                 name=f"kb{lt}_{eo}")
                    pk3 = pk[:].rearrange("p (h e) -> p h e", h=8)
                    nc.scalar.activation(kb[:, :, 0:64], pk3,
                                         AF.Relu, scale=scol[:, lt, 0:1])
                    nc.scalar.activation(kb[:, :, 64:128], pk3,
                                         AF.Relu, scale=scol[:, lt, 1:2])
                    # v copy on ACT into 65-wide layout; ones col on DVE
                    vb = wp.tile([128, 8, 65], bf16, tag="vb", bufs=3,
                                 name=f"vb{lt}_{eo}")
                    nc.scalar.activation(vb[:, :, 0:64],
                                         pv[:].rearrange("p (h e) -> p h e", h=8),
                                         AF.Copy)
                    nc.vector.memset(vb[:, :, 64:65], 1.0)
                    for hh in range(8):
                        h = eo * 8 + hh
                        nc.tensor.matmul(kv_ps[h // 4][:, h % 4, :],
                                         kb[:, hh, :], vb[:, hh, :],
                                         start=False, stop=(lt == LT - 1))
            for g in range(4):
                nc.vector.tensor_copy(kv_sb[g][:], kv_ps[g][:])
            if DBG:
                for g in range(4):
                    nc.gpsimd.dma_start(
                        dkv_d[:, g * 260:(g + 1) * 260],
                        kv_sb[g][:].rearrange("p a b -> p (a b)"))

            # ---- phase 2: q, attention, output projection ---------------
            # Emission order pipelines chunks: attn(lc) -> q-proj(lc+1) ->
            # out-proj(lc), so PE has q-projection matmuls to run while the
            # DVE/GpSimd z-chain of chunk lc drains.
            def build_q(lc):
                csl = slice(lc * LCW, (lc + 1) * LCW)
                qts = []
                for m in range(KT):
                    pq = pp.tile([128, LCW], f32, tag="A", bufs=2,
                                 name=f"pq{m}_{lc}")
                    for k in range(KT):
                        nc.tensor.matmul(pq[:], wq[k][:, m * 128:(m + 1) * 128],
                                         xt[k][:, csl],
                                         start=(k == 0), stop=(k == KT - 1))
                    qr = wp.tile([128, LCW], bf16, tag="qr", bufs=3,
                                 name=f"qr{m}_{lc}")
                    nc.scalar.activation(qr[:], pq[:], AF.Relu)
                    for j in range(2):
                        h = 2 * m + j
                        q_h = wp.tile([128, LCW], bf16, tag=f"qt{h}",
                                      name=f"qt{h}_{lc}")
                        rows = slice(j * 64, j * 64 + 64)
                        sin_src = (srow if j == 0 else srow2)[rows, csl]
                        cos_src = (srow2 if j == 0 else srow)[rows, csl]
                        nc.vector.tensor_tensor(q_h[0:64, :], qr[rows, :],
                                                sin_src, AL.mult)
                        nc.vector.tensor_tensor(q_h[64:128, :], qr[rows, :],
                                                cos_src, AL.mult)
                        qts.append(q_h)
                return qts

            qt = build_q(0)
            for lc in range(LC):
                at = [wp.tile([128, LCW], bf16, tag=f"at{m}", name=f"at{m}_{lc}",
                              bufs=2) for m in range(KT)]
                for h in range(H):
                    pa = pp.tile([65, LCW], f32, tag=f"kvp{h % 4}",
                                 name=f"pa{h}_{lc}")
                    nc.tensor.matmul(pa[:], kv_sb[h // 4][:, h % 4, :], qt[h][:],
                                     start=True, stop=True)
                    zr = wp.tile([1, LCW], f32, tag="zr", bufs=8,
                                 name=f"zr{h}_{lc}")
                    # z-denominator = x + eps instead of max(x, eps): x >= 0,
                    # and where they differ (x ~ eps) the numerator is ~0.
                    nc.scalar.activation(zr[:], pa[64:65, :], AF.Identity,
                                         bias=epsc[64:65, :])
                    nc.vector.reciprocal_approx_fast(zr[:], zr[:])
                    zb = wp.tile([64, LCW], f32, tag="zb", bufs=8,
                                 name=f"zb{h}_{lc}")
                    nc.gpsimd.partition_broadcast(zb[:], zr[:])
                    rows = slice((h % 2) * 64, (h % 2) * 64 + 64)
                    nc.vector.tensor_tensor(at[h // 2][rows, :], pa[0:64, :],
                                            zb[:], AL.mult)
                if lc + 1 < LC:
                    qt = build_q(lc + 1)
                for ltl in range(4):
                    lt = lc * 4 + ltl
                    tsl = slice(ltl * 128, (ltl + 1) * 128)
                    for eo in range(2):
                        esl = slice(eo * 512, (eo + 1) * 512)
                        po = pp.tile([128, 512], f32, tag="B", bufs=2,
                                     name=f"po{lt}_{eo}")
                        for m in range(KT):
                            nc.tensor.matmul(po[:], at[m][:, tsl], wo[m][:, esl],
                                             start=(m == 0), stop=(m == KT - 1))
                        ob = wp.tile([128, 512], f32, tag="ob", bufs=3,
                                     name=f"ob{lt}_{eo}")
                        nc.scalar.activation(ob[:], po[:], AF.Copy)
                        nc.gpsimd.dma_start(
                            out_d[lt * 128:(lt + 1) * 128, esl], ob[:])

    nc.compile()
    _CACHE["nc"] = nc
    return nc


def _prep_inputs(query, Wq, Wk, Wv, Wo):
    idx = (np.pi / 2) * np.arange(1, L + 1, dtype=np.float64) / L
    sin = np.sin(idx).astype(np.float32)
    cos = np.cos(idx).astype(np.float32)
    # scol[p, t, c]: c=0 sin, c=1 cos at l = t*128+p
    scol = np.stack([sin.reshape(LT, 128).T, cos.reshape(LT, 128).T],
                    axis=2).reshape(128, LT * 2).copy()
    srow = np.concatenate([np.tile(sin[None, :], (64, 1)),
                           np.tile(cos[None, :], (64, 1))], axis=0).astype(BF)
    srow2 = np.concatenate([srow[64:128], srow[0:64]]).copy()

    ws = {n: np.ascontiguousarray(w.T).astype(BF)
          for n, w in (("wq", Wq), ("wk", Wk), ("wv", Wv), ("wo", Wo))}
    in_maps = []
    for b in range(NB):
        m = dict(ws)
        m["xt"] = np.ascontiguousarray(query[:, b, :].T).astype(BF)
        m["scol"] = scol
        m["srow"] = srow
        m["srow2"] = srow2
        in_maps.append(m)
    return in_maps


def kernel(query, Wq, Wk, Wv, Wo, _trace=False, _trace_kwargs=None):
    nc = _build()
    in_maps = _prep_inputs(np.asarray(query, np.float32), Wq, Wk, Wv, Wo)
    res = run_bass_kernel_spmd(nc, in_maps, list(range(NB)), trace=_trace,
                               **(_trace_kwargs or {}))
    out = np.stack([res.results[b]["out"] for b in range(NB)], axis=1)
    if _trace:
        kernel.last_result = res
    return np.ascontiguousarray(out, dtype=np.float32)


# revision 16
# speedup vs baseline: 1.4442x; 1.0013x over previous
"""cosFormer non-causal linear attention on 8 trn2 NeuronCores.

Data-parallel over batch N=8: core b computes batch element b end-to-end.
Per core (L=2048, E=1024, H=16 heads, d=64):
  q = relu(x @ Wq.T), k = relu(x @ Wk.T), v = x @ Wv.T
  q_ = [q*sin, q*cos], k_ = [k*sin, k*cos]    (per-position cos/sin reweight)
  kv_h = k_h^T @ v_h, ksum_h = k_h^T @ 1      (accumulated over L in PSUM)
  attn_h^T = [kv_h | ksum_h]^T @ q_h^T, rows 0:64 scaled by
  z = 1/max(row 64, eps)
  out = attn @ Wo.T

Layouts: activations contract over E_in, so x and all weights enter
transposed (host-side). k_, v live L-major; q_, attn^T live feature-major,
which feeds the output projection without any transposes on device.
All matmuls bf16 with fp32 PSUM accumulation.

PSUM plan (8 banks): A(3 bufs): pk/pq/po, B(2): pv/pa, kvA(1), kvB(1),
ksum(1). kv/ksum accumulate across all of phase 1 with start=False
(banks pre-cleared by a K=1 zero matmul: start=True clears has_written
for the WHOLE bank, so slices sharing a bank must never issue starts).
"""

import numpy as np
import ml_dtypes

import concourse.bass as bass
import concourse.tile as tile
from concourse import bacc, mybir
from concourse.bass_utils import run_bass_kernel_spmd

L, NB, E, H, D = 2048, 8, 1024, 16, 64
KT = E // 128          # 8 contraction tiles
LC = 4                 # L chunks of 512
LCW = L // LC          # 512
LT = L // 128          # 16 l-tiles
EPS = 1e-6

f32 = mybir.dt.float32
bf16 = mybir.dt.bfloat16
AL = mybir.AluOpType
AF = mybir.ActivationFunctionType
BF = np.dtype(ml_dtypes.bfloat16)

_CACHE = {}


def _build():
    if "nc" in _CACHE:
        return _CACHE["nc"]
    nc = bacc.Bacc()

    xt_d = nc.declare_dram_parameter("xt", [E, L], bf16, isOutput=False)
    wq_d = nc.declare_dram_parameter("wq", [E, E], bf16, isOutput=False)
    wk_d = nc.declare_dram_parameter("wk", [E, E], bf16, isOutput=False)
    wv_d = nc.declare_dram_parameter("wv", [E, E], bf16, isOutput=False)
    wo_d = nc.declare_dram_parameter("wo", [E, E], bf16, isOutput=False)
    scol_d = nc.declare_dram_parameter("scol", [128, LT * 2], f32, isOutput=False)
    srow_d = nc.declare_dram_parameter("srow", [128, L], bf16, isOutput=False)
    srow2_d = nc.declare_dram_parameter("srow2", [128, L], bf16, isOutput=False)
    out_d = nc.declare_dram_parameter("out", [L, E], f32, isOutput=True)
    DBG = _CACHE.get("debug", False)
    if DBG:
        dkv_d = nc.declare_dram_parameter("dkv", [128, 16 * 65], f32, isOutput=True)

    with tile.TileContext(nc) as tc:
        with (
            tc.tile_pool(name="const", bufs=1) as cp,
            tc.tile_pool(name="work", bufs=1) as wp,
            tc.tile_pool(name="ps", bufs=1, space="PSUM") as pp,
        ):
            # ---- resident constants -------------------------------------
            # DMA order matches first-use order: scol first (first ACT),
            # then per k: wk_k, xt_k, wv_k so projection matmul k can start
            # as soon as its own operands land.
            scol = cp.tile([128, LT, 2], f32, tag="scol")
            nc.sync.dma_start(scol[:], scol_d.rearrange("p (t c) -> p t c", c=2)[:])
            xt, wk, wv, wq, wo = [], [], [], [], []
            res = {n: d.rearrange("(t p) e -> t p e", p=128)
                   for n, d in (("wk", wk_d), ("wv", wv_d), ("wq", wq_d),
                                ("wo", wo_d))}
            xt_re = xt_d.rearrange("(t p) l -> t p l", p=128)
            for k in range(KT):
                tw = cp.tile([128, E], bf16, tag=f"wk{k}", name=f"wk{k}")
                nc.sync.dma_start(tw[:], res["wk"][k])
                wk.append(tw)
                t = cp.tile([128, L], bf16, tag=f"xt{k}", name=f"xt{k}")
                nc.sync.dma_start(t[:], xt_re[k])
                xt.append(t)
            for k in range(KT):
                tv = cp.tile([128, E], bf16, tag=f"wv{k}", name=f"wv{k}")
                nc.sync.dma_start(tv[:], res["wv"][k])
                wv.append(tv)
            for k in range(KT):
                tq = cp.tile([128, E], bf16, tag=f"wq{k}", name=f"wq{k}")
                nc.sync.dma_start(tq[:], res["wq"][k])
                wq.append(tq)
            for k in range(KT):
                to = cp.tile([128, E], bf16, tag=f"wo{k}", name=f"wo{k}")
                nc.sync.dma_start(to[:], res["wo"][k])
                wo.append(to)
            srow = cp.tile([128, L], bf16, tag="srow")
            srow2 = cp.tile([128, L], bf16, tag="srow2")
            nc.sync.dma_start(srow[:], srow_d[:])
            nc.sync.dma_start(srow2[:], srow2_d[:])
            ones = cp.tile([128, 1], bf16, tag="ones")
            nc.vector.memset(ones[:], 1.0)
            epsc = cp.tile([128, 1], f32, tag="epsc")
            nc.vector.memset(epsc[:], EPS)

            # kv lhsT tiles in SBUF: 4 groups of 4 heads, (128=2d, 4, 65)
            kv_sb = [wp.tile([128, 4, 65], bf16, tag=f"kv{g}", name=f"kv{g}")
                     for g in range(4)]

            # ---- psum accumulators for phase 1 --------------------------
            kv_ps = [pp.tile([128, 4, 65], f32, tag=f"kvp{g}", name=f"kvp{g}")
                     for g in range(4)]
            zl = wp.tile([1, 128], bf16, tag="zl")
            zr5 = wp.tile([1, 260], bf16, tag="zr5")
            nc.vector.memset(zl[:], 0.0)
            nc.vector.memset(zr5[:], 0.0)
            for g in range(4):
                nc.tensor.matmul(kv_ps[g][:].rearrange("p a c -> p (a c)"),
                                 zl[:], zr5[:], start=True, stop=True)

            # ---- phase 1: kv/ksum accumulation --------------------------
            for lt in range(LT):
                lsl = slice(lt * 128, (lt + 1) * 128)
                for eo in range(2):
                    esl = slice(eo * 512, (eo + 1) * 512)
                    pk = pp.tile([128, 512], f32, tag="A", bufs=2, name=f"pk{lt}_{eo}")
                    pv = pp.tile([128, 512], f32, tag="B", bufs=2, name=f"pv{lt}_{eo}")
                    for k in range(KT):
                        nc.tensor.matmul(pk[:], xt[k][:, lsl], wk[k][:, esl],
                                         start=(k == 0), stop=(k == KT - 1))
                    for k in range(KT):
                        nc.tensor.matmul(pv[:], xt[k][:, lsl], wv[k][:, esl],
                                         start=(k == 0), stop=(k == KT - 1))
                    # k_ build on ACT: per head [0:64]=relu(k)*sin, [64:128]=relu(k)*cos
                    kb = wp.tile([128, 8, 128], bf16, tag="kb", bufs=3,
                                 name=f"kb{lt}_{eo}")
                    pk3 = pk[:].rearrange("p (h e) -> p h e", h=8)
                    nc.scalar.activation(kb[:, :, 0:64], pk3,
                                         AF.Relu, scale=scol[:, lt, 0:1])
                    nc.scalar.activation(kb[:, :, 64:128], pk3,
                                         AF.Relu, scale=scol[:, lt, 1:2])
                    # v copy on ACT into 65-wide layout; ones col on DVE
                    vb = wp.tile([128, 8, 65], bf16, tag="vb", bufs=3,
                                 name=f"vb{lt}_{eo}")
                    nc.scalar.activation(vb[:, :, 0:64],
                                         pv[:].rearrange("p (h e) -> p h e", h=8),
                                         AF.Copy)
                    nc.vector.memset(vb[:, :, 64:65], 1.0)
                    for hh in range(8):
                        h = eo * 8 + hh
                        nc.tensor.matmul(kv_ps[h // 4][:, h % 4, :],
                                         kb[:, hh, :], vb[:, hh, :],
                                         start=False, stop=(lt == LT - 1))
            for g in range(4):
                nc.vector.tensor_copy(kv_sb[g][:], kv_ps[g][:])
            if DBG:
                for g in range(4):
                    nc.gpsimd.dma_start(
                        dkv_d[:, g * 260:(g + 1) * 260],
                        kv_sb[g][:].rearrange("p a b -> p (a b)"))

            # ---- phase 2: q, attention, output projection ---------------
            # Emission order pipelines chunks: attn(lc) -> q-proj(lc+1) ->
            # out-proj(lc), so PE has q-projection matmuls to run while the
            # DVE/GpSimd z-chain of chunk lc drains.
            def build_q(lc):
                csl = slice(lc * LCW, (lc + 1) * LCW)
                qts = []
                for m in range(KT):
                    pq = pp.tile([128, LCW], f32, tag="A", bufs=2,
                                 name=f"pq{m}_{lc}")
                    for k in range(KT):
                        nc.tensor.matmul(pq[:], wq[k][:, m * 128:(m + 1) * 128],
                                         xt[k][:, csl],
                                         start=(k == 0), stop=(k == KT - 1))
                    qr = wp.tile([128, LCW], bf16, tag="qr", bufs=3,
                                 name=f"qr{m}_{lc}")
                    nc.scalar.activation(qr[:], pq[:], AF.Relu)
                    for j in range(2):
                        h = 2 * m + j
                        q_h = wp.tile([128, LCW], bf16, tag=f"qt{h}",
                                      name=f"qt{h}_{lc}")
                        rows = slice(j * 64, j * 64 + 64)
                        sin_src = (srow if j == 0 else srow2)[rows, csl]
                        cos_src = (srow2 if j == 0 else srow)[rows, csl]
                        nc.vector.tensor_tensor(q_h[0:64, :], qr[rows, :],
                                                sin_src, AL.mult)
                        nc.vector.tensor_tensor(q_h[64:128, :], qr[rows, :],
                                                cos_src, AL.mult)
                        qts.append(q_h)
                return qts

            qt = build_q(0)
            for lc in range(LC):
                at = [wp.tile([128, LCW], bf16, tag=f"at{m}", name=f"at{m}_{lc}",
                              bufs=3) for m in range(KT)]
                for h in range(H):
                    pa = pp.tile([65, LCW], f32, tag=f"kvp{h % 4}",
                                 name=f"pa{h}_{lc}")
                    nc.tensor.matmul(pa[:], kv_sb[h // 4][:, h % 4, :], qt[h][:],
                                     start=True, stop=True)
                    zr = wp.tile([1, LCW], f32, tag="zr", bufs=8,
                                 name=f"zr{h}_{lc}")
                    # z-denominator = x + eps instead of max(x, eps): x >= 0,
                    # and where they differ (x ~ eps) the numerator is ~0.
                    nc.scalar.activation(zr[:], pa[64:65, :], AF.Identity,
                                         bias=epsc[64:65, :])
                    nc.vector.reciprocal_approx_fast(zr[:], zr[:])
                    zb = wp.tile([64, LCW], f32, tag="zb", bufs=8,
                                 name=f"zb{h}_{lc}")
                    nc.gpsimd.partition_broadcast(zb[:], zr[:])
                    rows = slice((h % 2) * 64, (h % 2) * 64 + 64)
                    nc.vector.tensor_tensor(at[h // 2][rows, :], pa[0:64, :],
                                            zb[:], AL.mult)
                if lc + 1 < LC:
                    qt = build_q(lc + 1)
                for ltl in range(4):
                    lt = lc * 4 + ltl
                    tsl = slice(ltl * 128, (ltl + 1) * 128)
                    for eo in range(2):
                        esl = slice(eo * 512, (eo + 1) * 512)
                        po = pp.tile([128, 512], f32, tag="B", bufs=2,
                                     name=f"po{lt}_{eo}")
                        for m in range(KT):
                            nc.tensor.matmul(po[:], at[m][:, tsl], wo[m][:, esl],
                                             start=(m == 0), stop=(m == KT - 1))
                        ob = wp.tile([128, 512], f32, tag="ob", bufs=3,
                                     name=f"ob{lt}_{eo}")
                        nc.scalar.activation(ob[:], po[:], AF.Copy)
                        nc.gpsimd.dma_start(
                            out_d[lt * 128:(lt + 1) * 128, esl], ob[:])

    nc.compile()
    _CACHE["nc"] = nc
    return nc


def _prep_inputs(query, Wq, Wk, Wv, Wo):
    idx = (np.pi / 2) * np.arange(1, L + 1, dtype=np.float64) / L
    sin = np.sin(idx).astype(np.float32)
    cos = np.cos(idx).astype(np.float32)
    # scol[p, t, c]: c=0 sin, c=1 cos at l = t*128+p
    scol = np.stack([sin.reshape(LT, 128).T, cos.reshape(LT, 128).T],
                    axis=2).reshape(128, LT * 2).copy()
    srow = np.concatenate([np.tile(sin[None, :], (64, 1)),
                           np.tile(cos[None, :], (64, 1))], axis=0).astype(BF)
    srow2 = np.concatenate([srow[64:128], srow[0:64]]).copy()

    ws = {n: np.ascontiguousarray(w.T).astype(BF)
          for n, w in (("wq", Wq), ("wk", Wk), ("wv", Wv), ("wo", Wo))}
    in_maps = []
    for b in range(NB):
        m = dict(ws)
        m["xt"] = np.ascontiguousarray(query[:, b, :].T).astype(BF)
        m["scol"] = scol
        m["srow"] = srow
        m["srow2"] = srow2
        in_maps.append(m)
    return in_maps


def kernel(query, Wq, Wk, Wv, Wo, _trace=False, _trace_kwargs=None):
    nc = _build()
    in_maps = _prep_inputs(np.asarray(query, np.float32), Wq, Wk, Wv, Wo)
    res = run_bass_kernel_spmd(nc, in_maps, list(range(NB)), trace=_trace,
                               **(_trace_kwargs or {}))
    out = np.stack([res.results[b]["out"] for b in range(NB)], axis=1)
    if _trace:
        kernel.last_result = res
    return np.ascontiguousarray(out, dtype=np.float32)


# revision 17
# speedup vs baseline: 1.4707x; 1.0183x over previous
"""cosFormer non-causal linear attention on 8 trn2 NeuronCores.

Data-parallel over batch N=8: core b computes batch element b end-to-end.
Per core (L=2048, E=1024, H=16 heads, d=64):
  q = relu(x @ Wq.T), k = relu(x @ Wk.T), v = x @ Wv.T
  q_ = [q*sin, q*cos], k_ = [k*sin, k*cos]    (per-position cos/sin reweight)
  kv_h = k_h^T @ v_h, ksum_h = k_h^T @ 1      (accumulated over L in PSUM)
  attn_h^T = [kv_h | ksum_h]^T @ q_h^T, rows 0:64 scaled by
  z = 1/max(row 64, eps)
  out = attn @ Wo.T

Layouts: activations contract over E_in, so x and all weights enter
transposed (host-side). k_, v live L-major; q_, attn^T live feature-major,
which feeds the output projection without any transposes on device.
All matmuls bf16 with fp32 PSUM accumulation.

PSUM plan (8 banks): A(3 bufs): pk/pq/po, B(2): pv/pa, kvA(1), kvB(1),
ksum(1). kv/ksum accumulate across all of phase 1 with start=False
(banks pre-cleared by a K=1 zero matmul: start=True clears has_written
for the WHOLE bank, so slices sharing a bank must never issue starts).
"""

import numpy as np
import ml_dtypes

import concourse.bass as bass
import concourse.tile as tile
from concourse import bacc, mybir
from concourse.bass_utils import run_bass_kernel_spmd

L, NB, E, H, D = 2048, 8, 1024, 16, 64
KT = E // 128          # 8 contraction tiles
LC = 4                 # L chunks of 512
LCW = L // LC          # 512
LT = L // 128          # 16 l-tiles
EPS = 1e-6

f32 = mybir.dt.float32
bf16 = mybir.dt.bfloat16
AL = mybir.AluOpType
AF = mybir.ActivationFunctionType
BF = np.dtype(ml_dtypes.bfloat16)

_CACHE = {}


def _build():
    if "nc" in _CACHE:
        return _CACHE["nc"]
    nc = bacc.Bacc()

    xt_d = nc.declare_dram_parameter("xt", [E, L], bf16, isOutput=False)
    wq_d = nc.declare_dram_parameter("wq", [E, E], bf16, isOutput=False)
    wk_d = nc.declare_dram_parameter("wk", [E, E], bf16, isOutput=False)
    wv_d = nc.declare_dram_parameter("wv", [E, E], bf16, isOutput=False)
    wo_d = nc.declare_dram_parameter("wo", [E, E], bf16, isOutput=False)
    scol_d = nc.declare_dram_parameter("scol", [128, LT * 2], f32, isOutput=False)
    srow_d = nc.declare_dram_parameter("srow", [128, L], bf16, isOutput=False)
    srow2_d = nc.declare_dram_parameter("srow2", [128, L], bf16, isOutput=False)
    out_d = nc.declare_dram_parameter("out", [L, E], f32, isOutput=True)
    DBG = _CACHE.get("debug", False)
    if DBG:
        dkv_d = nc.declare_dram_parameter("dkv", [128, 16 * 65], f32, isOutput=True)

    with tile.TileContext(nc) as tc:
        with (
            tc.tile_pool(name="const", bufs=1) as cp,
            tc.tile_pool(name="work", bufs=1) as wp,
            tc.tile_pool(name="ps", bufs=1, space="PSUM") as pp,
        ):
            # ---- resident constants -------------------------------------
            # DMA order matches first-use order: scol first (first ACT),
            # then per k: wk_k, xt_k, wv_k so projection matmul k can start
            # as soon as its own operands land.
            scol = cp.tile([128, LT, 2], f32, tag="scol")
            nc.sync.dma_start(scol[:], scol_d.rearrange("p (t c) -> p t c", c=2)[:])
            xt, wk, wv, wq, wo = [], [], [], [], []
            res = {n: d.rearrange("(t p) e -> t p e", p=128)
                   for n, d in (("wk", wk_d), ("wv", wv_d), ("wq", wq_d),
                                ("wo", wo_d))}
            xt_re = xt_d.rearrange("(t p) l -> t p l", p=128)
            for k in range(KT):
                tw = cp.tile([128, E], bf16, tag=f"wk{k}", name=f"wk{k}")
                nc.sync.dma_start(tw[:], res["wk"][k])
                wk.append(tw)
                t = cp.tile([128, L], bf16, tag=f"xt{k}", name=f"xt{k}")
                nc.sync.dma_start(t[:], xt_re[k])
                xt.append(t)
            for k in range(KT):
                tv = cp.tile([128, E], bf16, tag=f"wv{k}", name=f"wv{k}")
                nc.sync.dma_start(tv[:], res["wv"][k])
                wv.append(tv)
            for k in range(KT):
                tq = cp.tile([128, E], bf16, tag=f"wq{k}", name=f"wq{k}")
                nc.sync.dma_start(tq[:], res["wq"][k])
                wq.append(tq)
            for k in range(KT):
                to = cp.tile([128, E], bf16, tag=f"wo{k}", name=f"wo{k}")
                nc.sync.dma_start(to[:], res["wo"][k])
                wo.append(to)
            srow = cp.tile([128, L], bf16, tag="srow")
            srow2 = cp.tile([128, L], bf16, tag="srow2")
            nc.sync.dma_start(srow[:], srow_d[:])
            nc.sync.dma_start(srow2[:], srow2_d[:])
            ones = cp.tile([128, 1], bf16, tag="ones")
            nc.vector.memset(ones[:], 1.0)
            epsc = cp.tile([128, 1], f32, tag="epsc")
            nc.vector.memset(epsc[:], EPS)

            # kv lhsT tiles in SBUF: 4 groups of 4 heads, (128=2d, 4, 65)
            kv_sb = [wp.tile([128, 4, 65], bf16, tag=f"kv{g}", name=f"kv{g}")
                     for g in range(4)]

            # ---- psum accumulators for phase 1 --------------------------
            kv_ps = [pp.tile([128, 4, 65], f32, tag=f"kvp{g}", name=f"kvp{g}")
                     for g in range(4)]
            zl = wp.tile([1, 128], bf16, tag="zl")
            zr5 = wp.tile([1, 260], bf16, tag="zr5")
            nc.vector.memset(zl[:], 0.0)
            nc.vector.memset(zr5[:], 0.0)
            for g in range(4):
                nc.tensor.matmul(kv_ps[g][:].rearrange("p a c -> p (a c)"),
                                 zl[:], zr5[:], start=True, stop=True)

            # ---- phase 1: kv/ksum accumulation --------------------------
            for lt in range(LT):
                lsl = slice(lt * 128, (lt + 1) * 128)
                for eo in range(2):
                    esl = slice(eo * 512, (eo + 1) * 512)
                    pk = pp.tile([128, 512], f32, tag="A", bufs=2, name=f"pk{lt}_{eo}")
                    pv = pp.tile([128, 512], f32, tag="B", bufs=2, name=f"pv{lt}_{eo}")
                    for k in range(KT):
                        nc.tensor.matmul(pk[:], xt[k][:, lsl], wk[k][:, esl],
                                         start=(k == 0), stop=(k == KT - 1))
                    for k in range(KT):
                        nc.tensor.matmul(pv[:], xt[k][:, lsl], wv[k][:, esl],
                                         start=(k == 0), stop=(k == KT - 1))
                    # k_ build on ACT: per head [0:64]=relu(k)*sin, [64:128]=relu(k)*cos
                    kb = wp.tile([128, 8, 128], bf16, tag="kb", bufs=4,
                                 name=f"kb{lt}_{eo}")
                    pk3 = pk[:].rearrange("p (h e) -> p h e", h=8)
                    nc.scalar.activation(kb[:, :, 0:64], pk3,
                                         AF.Relu, scale=scol[:, lt, 0:1])
                    nc.scalar.activation(kb[:, :, 64:128], pk3,
                                         AF.Relu, scale=scol[:, lt, 1:2])
                    # v copy on ACT into 65-wide layout; ones col on DVE
                    vb = wp.tile([128, 8, 65], bf16, tag="vb", bufs=4,
                                 name=f"vb{lt}_{eo}")
                    nc.scalar.activation(vb[:, :, 0:64],
                                         pv[:].rearrange("p (h e) -> p h e", h=8),
                                         AF.Copy)
                    nc.vector.memset(vb[:, :, 64:65], 1.0)
                    for hh in range(8):
                        h = eo * 8 + hh
                        nc.tensor.matmul(kv_ps[h // 4][:, h % 4, :],
                                         kb[:, hh, :], vb[:, hh, :],
                                         start=False, stop=(lt == LT - 1))
            for g in range(4):
                nc.vector.tensor_copy(kv_sb[g][:], kv_ps[g][:])
            if DBG:
                for g in range(4):
                    nc.gpsimd.dma_start(
                        dkv_d[:, g * 260:(g + 1) * 260],
                        kv_sb[g][:].rearrange("p a b -> p (a b)"))

            # ---- phase 2: q, attention, output projection ---------------
            # Emission order pipelines chunks: attn(lc) -> q-proj(lc+1) ->
            # out-proj(lc), so PE has q-projection matmuls to run while the
            # DVE/GpSimd z-chain of chunk lc drains.
            def build_q(lc):
                csl = slice(lc * LCW, (lc + 1) * LCW)
                qts = []
                for m in range(KT):
                    pq = pp.tile([128, LCW], f32, tag="A", bufs=2,
                                 name=f"pq{m}_{lc}")
                    for k in range(KT):
                        nc.tensor.matmul(pq[:], wq[k][:, m * 128:(m + 1) * 128],
                                         xt[k][:, csl],
                                         start=(k == 0), stop=(k == KT - 1))
                    qr = wp.tile([128, LCW], bf16, tag="qr", bufs=4,
                                 name=f"qr{m}_{lc}")
                    nc.scalar.activation(qr[:], pq[:], AF.Relu)
                    for j in range(2):
                        h = 2 * m + j
                        q_h = wp.tile([128, LCW], bf16, tag=f"qt{h}",
                                      name=f"qt{h}_{lc}")
                        rows = slice(j * 64, j * 64 + 64)
                        sin_src = (srow if j == 0 else srow2)[rows, csl]
                        cos_src = (srow2 if j == 0 else srow)[rows, csl]
                        nc.vector.tensor_tensor(q_h[0:64, :], qr[rows, :],
                                                sin_src, AL.mult)
                        nc.vector.tensor_tensor(q_h[64:128, :], qr[rows, :],
                                                cos_src, AL.mult)
                        qts.append(q_h)
                return qts

            qt = build_q(0)
            for lc in range(LC):
                at = [wp.tile([128, LCW], bf16, tag=f"at{m}", name=f"at{m}_{lc}",
                              bufs=3) for m in range(KT)]
                for h in range(H):
                    pa = pp.tile([65, LCW], f32, tag=f"kvp{h % 4}",
                                 name=f"pa{h}_{lc}")
                    nc.tensor.matmul(pa[:], kv_sb[h // 4][:, h % 4, :], qt[h][:],
                                     start=True, stop=True)
                    zr = wp.tile([1, LCW], f32, tag="zr", bufs=8,
                                 name=f"zr{h}_{lc}")
                    # z-denominator = x + eps instead of max(x, eps): x >= 0,
                    # and where they differ (x ~ eps) the numerator is ~0.
                    nc.scalar.activation(zr[:], pa[64:65, :], AF.Identity,
                                         bias=epsc[64:65, :])
                    nc.vector.reciprocal_approx_fast(zr[:], zr[:])
                    zb = wp.tile([64, LCW], f32, tag="zb", bufs=8,
                                 name=f"zb{h}_{lc}")
                    nc.gpsimd.partition_broadcast(zb[:], zr[:])
                    rows = slice((h % 2) * 64, (h % 2) * 64 + 64)
                    nc.vector.tensor_tensor(at[h // 2][rows, :], pa[0:64, :],
                                            zb[:], AL.mult)
                if lc + 1 < LC:
                    qt = build_q(lc + 1)
                for ltl in range(4):
                    lt = lc * 4 + ltl
                    tsl = slice(ltl * 128, (ltl + 1) * 128)
                    for eo in range(2):
                        esl = slice(eo * 512, (eo + 1) * 512)
                        po = pp.tile([128, 512], f32, tag="B", bufs=2,
                                     name=f"po{lt}_{eo}")
                        for m in range(KT):
                            nc.tensor.matmul(po[:], at[m][:, tsl], wo[m][:, esl],
                                             start=(m == 0), stop=(m == KT - 1))
                        ob = wp.tile([128, 512], f32, tag="ob", bufs=3,
                                     name=f"ob{lt}_{eo}")
                        nc.scalar.activation(ob[:], po[:], AF.Copy)
                        nc.gpsimd.dma_start(
                            out_d[lt * 128:(lt + 1) * 128, esl], ob[:])

    nc.compile()
    _CACHE["nc"] = nc
    return nc


def _prep_inputs(query, Wq, Wk, Wv, Wo):
    idx = (np.pi / 2) * np.arange(1, L + 1, dtype=np.float64) / L
    sin = np.sin(idx).astype(np.float32)
    cos = np.cos(idx).astype(np.float32)
    # scol[p, t, c]: c=0 sin, c=1 cos at l = t*128+p
    scol = np.stack([sin.reshape(LT, 128).T, cos.reshape(LT, 128).T],
                    axis=2).reshape(128, LT * 2).copy()
    srow = np.concatenate([np.tile(sin[None, :], (64, 1)),
                           np.tile(cos[None, :], (64, 1))], axis=0).astype(BF)
    srow2 = np.concatenate([srow[64:128], srow[0:64]]).copy()

    ws = {n: np.ascontiguousarray(w.T).astype(BF)
          for n, w in (("wq", Wq), ("wk", Wk), ("wv", Wv), ("wo", Wo))}
    in_maps = []
    for b in range(NB):
        m = dict(ws)
        m["xt"] = np.ascontiguousarray(query[:, b, :].T).astype(BF)
        m["scol"] = scol
        m["srow"] = srow
        m["srow2"] = srow2
        in_maps.append(m)
    return in_maps


def kernel(query, Wq, Wk, Wv, Wo, _trace=False, _trace_kwargs=None):
    nc = _build()
    in_maps = _prep_inputs(np.asarray(query, np.float32), Wq, Wk, Wv, Wo)
    res = run_bass_kernel_spmd(nc, in_maps, list(range(NB)), trace=_trace,
                               **(_trace_kwargs or {}))
    out = np.stack([res.results[b]["out"] for b in range(NB)], axis=1)
    if _trace:
        kernel.last_result = res
    return np.ascontiguousarray(out, dtype=np.float32)
